# revision 1
# baseline (speedup 1.0000x reference)
"""MoE top-2-of-8 SwiGLU feed-forward on 8 Trainium2 NeuronCores.

Strategy: expert-parallel, pipelined over two 4096-token halves.
 - Router: core c routes tokens [c*1024,(c+1)*1024) in full fp32 on the PE
   (top-2 selection must match the reference bit-for-bit; the smallest
   top2/top3 logit gap in this data is ~6e-5, far above fp32 matmul error),
   then the combine-weight table w[N,8] is AllGathered (32KB per core).
 - Dispatch (per half): core c builds on-device the compacted list of its
   expert's tokens via triangular-matmul prefix sums, and scatters each
   128-token tile's rows [x | w | token_id] into a DRAM staging buffer with
   one indirect DMA per tile. Invalid tokens fill the tail slots with
   id=N (a dump row) and w=0, so every staging slot is defined.
 - FFN (per half): h=x@W1+b1, g=x@Wg+bg, y=(silu(h)*g)@W2+b2 over <=1280
   gathered tokens in float32r (~1e-4 rel err), weight-stationary over
   [512,512,256] token blocks, h/g/y on alternating PSUM bank sets, PSUM
   evictions (+bias) on the Scalar engine.
 - Combine: y rows are scaled by w and scattered to a token-indexed partial;
   ReduceScatter(add) per half gives each core its finished output rows.
   RS(half0) overlaps FFN(half1).
"""
import numpy as np

import concourse.bass as bass
import concourse.mybir as mybir
import concourse.tile as tile
from concourse.masks import make_identity
from concourse.vector_clock import ScopedClock

P = 128
N_CORES = 8
B, T, C, E = 4, 2048, 1024, 8
N = B * T                  # 8192 tokens
HALF = N // 2              # 4096 tokens per pipeline half
NQH = HALF // P            # 32 token tiles per half
SLICE = N // N_CORES       # 1024 tokens per core slice
NT_SL = SLICE // P         # 8 tiles per slice
CC = C // P                # 8 feature chunks
CAP_H = 1280               # per-half capacity of the gathered-token buffer
BLOCKS = (512, 512, 256)   # FFN token blocks per half
NST = CAP_H // P           # 10 sub-tiles per half
ROW = 1032                 # staging row: 1024 x + 1 w + 1 id + 6 pad
BIG = 1.0e6
FP32R = mybir.dt.float32r
F32 = mybir.dt.float32
I32 = mybir.dt.int32
ACTF = mybir.ActivationFunctionType

# ---------------------------------------------------------------- tile patch
# Walrus in this environment accepts only ONE semaphore wait per instruction.
# Tile attaches several (end-of-kernel drain, multi-producer deps). Split the
# extras onto same-engine NoOps/Drains placed immediately before.


def _drain_and_barrier(self, tick_clock, wait_clock):
    drain_inst = self.nc.sync.drain()
    wait_clock.add_sem_waits(
        drain_inst.ins, ScopedClock({None: tick_clock.global_clock})
    )
    si = drain_inst.ins.sync_info
    if si is not None and si.on_wait is not None and len(si.on_wait) > 1:
        waits = list(si.on_wait)
        si.on_wait = waits[:1]
        for w in waits[1:]:
            extra = self.nc.sync.drain()
            esi = extra.ins.sync_info
            if esi is None:
                esi = mybir.SyncInfo(on_wait=[], on_update=[])
                extra.ins.sync_info = esi
            esi.on_wait = [w]
    self.nc.all_engine_barrier()
    assert self.sems is not None
    popped = self.nc._tile_sem_poison_stack.pop()
    assert popped is self._sem_poison
    self.nc.clear_and_free_semaphores(list(self.sems.allocated().values()))
    self.nc.all_engine_barrier()


tile.TileContext._drain_and_barrier = _drain_and_barrier


def split_multi_waits(nc, max_waits=1):
    for f in nc.m.functions:
        for bb in f.blocks:
            new = []
            dirty = False
            for ins in bb.instructions:
                si = getattr(ins, "sync_info", None)
                if si is not None and si.on_wait and len(si.on_wait) > max_waits:
                    waits = list(si.on_wait)
                    extra, keep = waits[:-max_waits], waits[-max_waits:]
                    for j in range(0, len(extra), max_waits):
                        nop = mybir.InstNoOp(
                            name=f"{ins.name}-wsplit{j}", ins=[], outs=[]
                        )
                        nop.engine = ins.engine
                        nop.sync_info = mybir.SyncInfo(
                            on_wait=extra[j : j + max_waits], on_update=[]
                        )
                        new.append(nop)
                    si.on_wait = keep
                    dirty = True
                new.append(ins)
            if dirty:
                bb.instructions = new


# ---------------------------------------------------------------- kernel IR


def build_nc():
    nc = bass.Bass()
    x_in = nc.declare_dram_parameter("x", [N, C], F32, isOutput=False)
    xsl_in = nc.declare_dram_parameter("xsl", [SLICE, C], F32, isOutput=False)
    wr_in = nc.declare_dram_parameter("wr", [C, E], F32, isOutput=False)
    br_in = nc.declare_dram_parameter("br", [E], F32, isOutput=False)
    esel_in = nc.declare_dram_parameter("esel", [P, E], F32, isOutput=False)
    w1_in = nc.declare_dram_parameter("w1", [C, C], F32, isOutput=False)
    b1_in = nc.declare_dram_parameter("b1", [C], F32, isOutput=False)
    wg_in = nc.declare_dram_parameter("wg", [C, C], F32, isOutput=False)
    bg_in = nc.declare_dram_parameter("bg", [C], F32, isOutput=False)
    w2_in = nc.declare_dram_parameter("w2", [C, C], F32, isOutput=False)
    b2_in = nc.declare_dram_parameter("b2", [C], F32, isOutput=False)
    y_out = nc.declare_dram_parameter("y_slice", [SLICE, C], F32, isOutput=True)

    w_sl = nc.dram_tensor("w_sl", [P, NT_SL * E], F32)  # row p, col tt*8+e
    w_all = nc.dram_tensor("w_all", [N_CORES * P, NT_SL * E], F32, addr_space="Shared")
    sidetabs = [nc.dram_tensor(f"sidetab{h}", [CAP_H, 4], F32) for h in range(2)]
    partials = [nc.dram_tensor(f"partial{h}", [HALF, C], F32) for h in range(2)]
    rs_ints = [nc.dram_tensor(f"rs_int{h}", [HALF // N_CORES, C], F32) for h in range(2)]
    GROUPS = [list(range(N_CORES))]

    with tile.TileContext(nc) as tc:
        with tc.tile_pool(name="const", bufs=1) as cpool:
            breg_cap = nc.gpsimd.to_reg(CAP_H - 1)
            breg_half = nc.gpsimd.to_reg(HALF - 1)
            ident = cpool.tile([P, P], F32)
            make_identity(nc, ident[:])
            ones1 = cpool.tile([1, P], F32)
            nc.vector.memset(ones1[:], 1.0)
            ones128 = cpool.tile([P, P], F32)
            nc.vector.memset(ones128[:], 1.0)
            tri128 = cpool.tile([P, P], F32)
            nc.vector.memset(tri128[:], 1.0)
            nc.gpsimd.affine_select(
                out=tri128[:], in_=tri128[:], pattern=[[1, P]],
                compare_op=mybir.AluOpType.is_ge, fill=0.0,
                base=-1, channel_multiplier=-1)
            tri32 = cpool.tile([NQH, NQH], F32)
            nc.vector.memset(tri32[:], 1.0)
            nc.gpsimd.affine_select(
                out=tri32[:], in_=tri32[:], pattern=[[1, NQH]],
                compare_op=mybir.AluOpType.is_ge, fill=0.0,
                base=-1, channel_multiplier=-1)
            i32id = cpool.tile([NQH, NQH], F32)
            make_identity(nc, i32id[:])
            zeros = cpool.tile([P, C], F32)
            nc.vector.memset(zeros[:], 0.0)
            b1_sb = cpool.tile([P, CC], F32)
            nc.sync.dma_start(out=b1_sb[:], in_=b1_in.rearrange("(ic p) -> p ic", p=P))
            bg_sb = cpool.tile([P, CC], F32)
            nc.sync.dma_start(out=bg_sb[:], in_=bg_in.rearrange("(ic p) -> p ic", p=P))
            b2_sb = cpool.tile([P, CC], F32)
            nc.sync.dma_start(out=b2_sb[:], in_=b2_in.rearrange("(mc p) -> p mc", p=P))

            # ---------------- phase R: router over this core's slice -------
            with (
                tc.tile_pool(name="rpool", bufs=2) as rp,
                tc.tile_pool(name="rpsum", bufs=4, space="PSUM") as rps,
            ):
                wr_sb = rp.tile([P, CC, E], F32, name="wr_sb", bufs=1)
                nc.sync.dma_start(out=wr_sb[:], in_=wr_in.rearrange("(cc p) e -> p cc e", p=P))
                br_sb = rp.tile([1, E], F32, name="br_sb", bufs=1)
                nc.sync.dma_start(out=br_sb[:], in_=br_in[None, :])
                xT_all = rp.tile([P, CC, SLICE], F32, name="xT_all", bufs=1)
                lg_all = rp.tile([P, NT_SL, E], F32, name="lg_all", bufs=1)
                for tt in range(NT_SL):
                    xt = rp.tile([P, C], F32, name=f"xt{tt}", tag="xt")
                    nc.sync.dma_start(out=xt[:], in_=xsl_in[tt * P:(tt + 1) * P, :])
                    for cc in range(CC):
                        ps_t = rps.tile([P, P], F32, name=f"rt{tt}_{cc}", tag="ps_t")
                        nc.tensor.transpose(out=ps_t[:], in_=xt[:, cc * P:(cc + 1) * P],
                                            identity=ident[:])
                        nc.vector.tensor_copy(out=xT_all[:, cc, tt * P:(tt + 1) * P],
                                              in_=ps_t[:])
                for tt in range(NT_SL):
                    ps_log = rps.tile([P, E], F32, name=f"rl{tt}", tag="ps_log", bufs=2)
                    for cc in range(CC):
                        nc.tensor.matmul(out=ps_log[:],
                                         lhsT=xT_all[:, cc, tt * P:(tt + 1) * P],
                                         rhs=wr_sb[:, cc],
                                         start=(cc == 0), stop=False)
                    nc.tensor.matmul(out=ps_log[:], lhsT=ones1[:], rhs=br_sb[:],
                                     start=False, stop=True)
                    nc.vector.tensor_copy(out=lg_all[:, tt], in_=ps_log[:])
                # batched softmax + top-2 over all 8 tiles
                s8_all = rp.tile([P, NT_SL, 8], F32, name="s8_all", bufs=1)
                for tt in range(NT_SL):
                    nc.vector.max(out=s8_all[:, tt], in_=lg_all[:, tt])
                lsh = rp.tile([P, NT_SL, E], F32, name="lsh", bufs=1)
                nc.vector.tensor_tensor(out=lsh[:], in0=lg_all[:],
                                        in1=s8_all[:, :, 0:1].to_broadcast([P, NT_SL, E]),
                                        op=mybir.AluOpType.subtract)
                ex_all = rp.tile([P, NT_SL, E], F32, name="ex_all", bufs=1)
                nc.scalar.activation(ex_all[:], lsh[:], ACTF.Exp)
                ssum = rp.tile([P, NT_SL], F32, name="ssum", bufs=1)
                nc.vector.reduce_sum(out=ssum[:], in_=ex_all[:], axis=mybir.AxisListType.X)
                rec = rp.tile([P, NT_SL], F32, name="rec", bufs=1)
                nc.vector.reciprocal(rec[:], ssum[:])
                mk = rp.tile([P, NT_SL, E], F32, name="mk", bufs=1)
                nc.vector.tensor_tensor(out=mk[:], in0=lg_all[:],
                                        in1=s8_all[:, :, 1:2].to_broadcast([P, NT_SL, E]),
                                        op=mybir.AluOpType.is_ge)
                wt_all = rp.tile([P, NT_SL, E], F32, name="wt_all", bufs=1)
                nc.vector.tensor_tensor(out=wt_all[:], in0=ex_all[:],
                                        in1=rec[:].unsqueeze(2).to_broadcast([P, NT_SL, E]),
                                        op=mybir.AluOpType.mult)
                nc.vector.tensor_mul(wt_all[:], wt_all[:], mk[:])
                nc.sync.dma_start(out=w_sl.rearrange("p (tt e) -> p tt e", e=E),
                                  in_=wt_all[:])

            # zero the token-indexed partials + side-table templates on the
            # scalar DMA queue (emitted after the router so its Exp activation
            # is not queued behind them on the Scalar engine)
            tmpl = cpool.tile([P, NST, 4], F32)
            nc.vector.memset(tmpl[:], 0.0)
            nc.vector.memset(tmpl[:, :, 1], float(N - 1))
            nc.vector.memset(tmpl[:, :, 2], float(HALF))
            for h in range(2):
                nc.scalar.dma_start(
                    out=sidetabs[h].rearrange("(st p) c -> p st c", p=P), in_=tmpl[:])
            for h in range(2):
                for t in range(HALF // P):
                    nc.scalar.dma_start(out=partials[h][t * P:(t + 1) * P, :], in_=zeros[:])
            nc.gpsimd.collective_compute(
                "AllGather", mybir.AluOpType.bypass, replica_groups=GROUPS,
                ins=[w_sl[:]], outs=[w_all[:]],
            )

            # ---------------- phase D: dispatch (per half) ------------------
            dctx = [
                tc.tile_pool(name="dpool", bufs=1),
                tc.tile_pool(name="dpsum", bufs=1, space="PSUM"),
                tc.tile_pool(name="xwpool", bufs=8),
            ]
            dpool = dctx[0].__enter__()
            dps = dctx[1].__enter__()
            xp = dctx[2].__enter__()
            esel = dpool.tile([P, E], F32)
            nc.sync.dma_start(out=esel[:], in_=esel_in[:])

            slot_is = []
            sides = []
            for h in range(2):
                q0 = h * NQH
                w8 = dpool.tile([P, NQH, E], F32, name=f"w8_{h}", tag="w8")
                nc.sync.dma_start(
                    out=w8.rearrange("p (c tt) e -> p c tt e", tt=NT_SL),
                    in_=w_all[h * 4 * P:(h + 1) * 4 * P].rearrange(
                        "(c p) (tt e) -> p c tt e", p=P, e=E))
                wprod = dpool.tile([P, NQH, E], F32, name=f"wp_{h}", tag="wp")
                nc.vector.tensor_mul(
                    wprod[:], w8[:],
                    esel[:].unsqueeze(1).to_broadcast([P, NQH, E]))
                wcol = dpool.tile([P, NQH], F32, name=f"wc_{h}", tag="wc")
                nc.vector.reduce_sum(out=wcol[:], in_=wprod[:], axis=mybir.AxisListType.X)
                mask = dpool.tile([P, NQH], F32, name=f"dm_{h}", tag="dm")
                nc.vector.tensor_scalar(mask[:], wcol[:], 0.0, scalar2=None,
                                        op0=mybir.AluOpType.is_gt)
                onescol = dpool.tile([P, 1], F32, name=f"oc_{h}", tag="oc")
                nc.vector.memset(onescol[:], 1.0)
                rowsum = dpool.tile([P, 1], F32, name=f"rs_{h}", tag="rsm")
                nc.vector.reduce_sum(out=rowsum[:], in_=mask[:], axis=mybir.AxisListType.X)

                ps_tot = dps.tile([NQH, 1], F32, name=f"pt_{h}", tag="pt")
                nc.tensor.matmul(out=ps_tot[:], lhsT=mask[:], rhs=onescol[:],
                                 start=True, stop=True)
                t32 = dpool.tile([NQH, 1], F32, name=f"t32_{h}", tag="t32")
                nc.vector.tensor_copy(out=t32[:], in_=ps_tot[:])
                ps_b = dps.tile([NQH, 1], F32, name=f"pb_{h}", tag="pb")
                nc.tensor.matmul(out=ps_b[:], lhsT=tri32[:], rhs=t32[:],
                                 start=True, stop=True)
                b32 = dpool.tile([NQH, 1], F32, name=f"b32_{h}", tag="b32")
                nc.vector.tensor_copy(out=b32[:], in_=ps_b[:])
                ps_brow = dps.tile([1, NQH], F32, name=f"pr_{h}", tag="pr")
                nc.tensor.matmul(out=ps_brow[:], lhsT=b32[:], rhs=i32id[:],
                                 start=True, stop=True)
                brow = dpool.tile([1, NQH], F32, name=f"br_{h}", tag="brw")
                nc.vector.tensor_copy(out=brow[:], in_=ps_brow[:])
                ps_pos = dps.tile([P, NQH], F32, name=f"pp_{h}", tag="pp")
                nc.tensor.matmul(out=ps_pos[:], lhsT=tri128[:], rhs=mask[:],
                                 start=True, stop=False)
                nc.tensor.matmul(out=ps_pos[:], lhsT=ones1[:], rhs=brow[:],
                                 start=False, stop=True)
                ps_cnt = dps.tile([P, 1], F32, name=f"pc_{h}", tag="pc")
                nc.tensor.matmul(out=ps_cnt[:], lhsT=ones128[:], rhs=rowsum[:],
                                 start=True, stop=True)
                cnt = dpool.tile([P, 1], F32, name=f"cnt_{h}", tag="cnt")
                nc.vector.tensor_copy(out=cnt[:], in_=ps_cnt[:])
                pos = dpool.tile([P, NQH], F32, name=f"pos_{h}", tag="pos")
                nc.vector.tensor_copy(out=pos[:], in_=ps_pos[:])
                iota_i = dpool.tile([P, NQH], I32, name=f"ii_{h}", tag="ii")
                nc.gpsimd.iota(iota_i[:], pattern=[[P, NQH]], base=0,
                               channel_multiplier=1)
                iota_f = dpool.tile([P, NQH], F32, name=f"if_{h}", tag="if")
                nc.vector.tensor_copy(out=iota_f[:], in_=iota_i[:])
                m1m = dpool.tile([P, NQH], F32, name=f"m1m_{h}", tag="m1m")
                nc.vector.tensor_scalar(m1m[:], mask[:], -1.0, scalar2=1.0,
                                        op0=mybir.AluOpType.mult, op1=mybir.AluOpType.add)
                slotf = dpool.tile([P, NQH], F32, name=f"sf_{h}", tag="sf")
                nc.vector.tensor_mul(slotf[:], pos[:], mask[:])
                t_big = dpool.tile([P, NQH], F32, name=f"tb_{h}", tag="tb")
                nc.vector.tensor_scalar_mul(t_big[:], m1m[:], BIG)
                nc.vector.tensor_add(slotf[:], slotf[:], t_big[:])
                slot_i = dpool.tile([P, NQH], I32, name=f"si_{h}", tag="si")
                nc.vector.tensor_copy(out=slot_i[:], in_=slotf[:])
                # side-band columns: [w, gather id (global), scatter id (half-local)]
                idf = dpool.tile([P, NQH], F32, name=f"idf_{h}", tag="idf")
                nc.vector.tensor_scalar_add(idf[:], iota_f[:], float(h * HALF))
                side = dpool.tile([P, NQH, 4], F32, name=f"side_{h}", tag="side")
                nc.vector.tensor_copy(out=side[:, :, 0], in_=wcol[:])
                nc.vector.tensor_copy(out=side[:, :, 1], in_=idf[:])
                nc.vector.tensor_scalar_add(side[:, :, 2], iota_f[:], 0.0)
                for q in range(NQH):
                    st_ap = sidetabs[h][0:1, :]
                    st_ap = bass.AP(tensor=st_ap.tensor, offset=0, ap=st_ap.ap,
                                    dep_tracking_offset=q * 4)
                    nc.gpsimd.indirect_dma_start(
                        out=st_ap,
                        out_offset=bass.IndirectOffsetOnAxis(ap=slot_i[:, q:q + 1], axis=0),
                        in_=side[:, q, :], in_offset=None,
                        bounds_check=breg_cap, oob_is_err=False,
                    )
            for cm in reversed(dctx):
                cm.__exit__(None, None, None)

            # ---------------- phase F: expert FFN (per half) ---------------
            with (
                tc.tile_pool(name="fbig", bufs=1) as fbig,
                tc.tile_pool(name="fpool", bufs=3) as fp,
                tc.tile_pool(name="fw", bufs=3) as fw,
                tc.tile_pool(name="fsmall", bufs=2) as fs,
                tc.tile_pool(name="fpsum", bufs=1, space="PSUM") as fps,
            ):
                for h in range(2):
                    sv = fs.tile([P, NST, 4], F32, name=f"sv{h}", tag="sv")
                    nc.sync.dma_start(out=sv[:], in_=sidetabs[h].rearrange("(st p) c -> p st c", p=P))
                    wv = fs.tile([P, NST], F32, name=f"wv{h}", tag="wv")
                    nc.vector.tensor_copy(out=wv[:], in_=sv[:, :, 0])
                    idg = fs.tile([P, NST], I32, name=f"idg{h}", tag="idg")
                    nc.vector.tensor_copy(out=idg[:], in_=sv[:, :, 1])
                    idv = fs.tile([P, NST], I32, name=f"idv{h}", tag="idv")
                    nc.vector.tensor_copy(out=idv[:], in_=sv[:, :, 2])
                    xgT = fbig.tile([P, CC, CAP_H], FP32R, name=f"xgT{h}", tag="big")
                    pstag = ["mmA", "mmB"]
                    for st in range(NST):
                        xg = fp.tile([P, C], F32, name=f"xg_{h}_{st}", tag="xg", bufs=6)
                        nc.gpsimd.indirect_dma_start(
                            out=xg[:], out_offset=None,
                            in_=x_in[:],
                            in_offset=bass.IndirectOffsetOnAxis(ap=idg[:, st:st + 1], axis=0),
                        )
                        for cc in range(CC):
                            ps_t = fps.tile([P, P], F32, name=f"ft{h}_{st}_{cc}",
                                            tag="trA" if (st * CC + cc) % 2 == 0 else "trB")
                            nc.tensor.transpose(out=ps_t[:], in_=xg[:, cc * P:(cc + 1) * P],
                                                identity=ident[:])
                            nc.vector.tensor_copy(out=xgT[:, cc, st * P:(st + 1) * P],
                                                  in_=ps_t[:])
                    # L1
                    a_t = fbig.tile([P, CC, CAP_H], FP32R, name=f"a{h}", tag="abuf")
                    for ic in range(CC):
                        w1t = fw.tile([P, CC, P], F32, name=f"w1t{h}_{ic}", tag="wld")
                        nc.sync.dma_start(
                            out=w1t[:],
                            in_=w1_in.rearrange("(cc p) i -> p cc i", p=P)[:, :, ic * P:(ic + 1) * P])
                        w1r = fw.tile([P, CC, P], FP32R, name=f"w1r{h}_{ic}", tag="wldr")
                        nc.vector.tensor_copy(out=w1r[:], in_=w1t[:])
                        wgt = fw.tile([P, CC, P], F32, name=f"wgt{h}_{ic}", tag="wld2")
                        nc.sync.dma_start(
                            out=wgt[:],
                            in_=wg_in.rearrange("(cc p) i -> p cc i", p=P)[:, :, ic * P:(ic + 1) * P])
                        wgr = fw.tile([P, CC, P], FP32R, name=f"wgr{h}_{ic}", tag="wldr2")
                        nc.vector.tensor_copy(out=wgr[:], in_=wgt[:])

                        ps_h = [fps.tile([P, 512], F32, name=f"psh{h}_{ic}_{b}",
                                         tag=f"mmA{b}") for b in range(len(BLOCKS))]
                        bo = [0, 512, 1024]
                        for cc in range(CC):
                            for b, bw in enumerate(BLOCKS):
                                nc.tensor.matmul(out=ps_h[b][:, :bw], lhsT=w1r[:, cc],
                                                 rhs=xgT[:, cc, bo[b]:bo[b] + bw],
                                                 start=(cc == 0), stop=(cc == CC - 1))
                        sil = fs.tile([P, CAP_H], F32, name=f"sil{h}_{ic}", tag="sil")
                        for b, bw in enumerate(BLOCKS):
                            nc.scalar.activation(sil[:, bo[b]:bo[b] + bw], ps_h[b][:, :bw],
                                                 ACTF.Silu, bias=b1_sb[:, ic:ic + 1])
                        ps_g = [fps.tile([P, 512], F32, name=f"psg{h}_{ic}_{b}",
                                         tag=f"mmB{b}") for b in range(len(BLOCKS))]
                        for cc in range(CC):
                            for b, bw in enumerate(BLOCKS):
                                nc.tensor.matmul(out=ps_g[b][:, :bw], lhsT=wgr[:, cc],
                                                 rhs=xgT[:, cc, bo[b]:bo[b] + bw],
                                                 start=(cc == 0), stop=(cc == CC - 1))
                        g_sb = fs.tile([P, CAP_H], F32, name=f"g{h}_{ic}", tag="gsb")
                        for b, bw in enumerate(BLOCKS):
                            nc.scalar.activation(g_sb[:, bo[b]:bo[b] + bw], ps_g[b][:, :bw],
                                                 ACTF.Identity, bias=bg_sb[:, ic:ic + 1])
                        nc.vector.tensor_mul(a_t[:, ic], sil[:], g_sb[:])

                    # L2
                    y_tok = fbig.tile([P, NST, C], F32, name=f"ytok{h}", tag="big")
                    for mc in range(CC):
                        w2t = fw.tile([P, CC, P], F32, name=f"w2t{h}_{mc}", tag="wld")
                        nc.sync.dma_start(
                            out=w2t[:],
                            in_=w2_in.rearrange("(ic p) c -> p ic c", p=P)[:, :, mc * P:(mc + 1) * P])
                        w2r = fw.tile([P, CC, P], FP32R, name=f"w2r{h}_{mc}", tag="wldr")
                        nc.vector.tensor_copy(out=w2r[:], in_=w2t[:])
                        ps_y = [fps.tile([P, 512], F32, name=f"psy{h}_{mc}_{b}",
                                         tag=f"mm{'A' if mc % 2 == 0 else 'B'}{b}")
                                for b in range(len(BLOCKS))]
                        bo = [0, 512, 1024]
                        for ic in range(CC):
                            for b, bw in enumerate(BLOCKS):
                                nc.tensor.matmul(out=ps_y[b][:, :bw], lhsT=w2r[:, ic],
                                                 rhs=a_t[:, ic, bo[b]:bo[b] + bw],
                                                 start=(ic == 0), stop=(ic == CC - 1))
                        y_sb = fs.tile([P, CAP_H], F32, name=f"ysb{h}_{mc}", tag="ysb")
                        for b, bw in enumerate(BLOCKS):
                            nc.scalar.activation(y_sb[:, bo[b]:bo[b] + bw], ps_y[b][:, :bw],
                                                 ACTF.Identity, bias=b2_sb[:, mc:mc + 1])
                        for st in range(NST):
                            ps_t2 = fps.tile([P, P], F32, name=f"bt{h}_{mc}_{st}",
                                             tag="trA" if (st + mc) % 2 == 0 else "trB")
                            nc.tensor.transpose(out=ps_t2[:],
                                                in_=y_sb[:, st * P:(st + 1) * P],
                                                identity=ident[:])
                            nc.vector.tensor_copy(out=y_tok[:, st, mc * P:(mc + 1) * P],
                                                  in_=ps_t2[:])
                    for st in range(NST):
                        nc.vector.tensor_scalar_mul(y_tok[:, st], y_tok[:, st],
                                                    wv[:, st:st + 1])
                        pt_ap = partials[h][0:1, :]
                        pt_ap = bass.AP(tensor=pt_ap.tensor, offset=0, ap=pt_ap.ap,
                                        dep_tracking_offset=st * C)
                        nc.gpsimd.indirect_dma_start(
                            out=pt_ap,
                            out_offset=bass.IndirectOffsetOnAxis(
                                ap=idv[:, st:st + 1], axis=0),
                            in_=y_tok[:, st], in_offset=None,
                            bounds_check=breg_half, oob_is_err=False,
                        )
                    # combine this half while the next half computes
                    nc.gpsimd.collective_compute(
                        "ReduceScatter", mybir.AluOpType.add, replica_groups=GROUPS,
                        ins=[partials[h][0:HALF, :]], outs=[rs_ints[h][:]],
                    )

            # ---------------- output copy ----------------------------------
            with tc.tile_pool(name="opool", bufs=2) as op:
                HS = HALF // N_CORES  # 512 rows per half per core
                for h in range(2):
                    for t in range(HS // P):
                        ot = op.tile([P, C], F32, name=f"ot{h}_{t}", tag="ot")
                        nc.sync.dma_start(out=ot[:], in_=rs_ints[h][t * P:(t + 1) * P, :])
                        nc.sync.dma_start(
                            out=y_out[h * HS + t * P:h * HS + (t + 1) * P, :], in_=ot[:])

    split_multi_waits(nc)
    return nc


_NC_CACHE = None


def _get_nc():
    global _NC_CACHE
    if _NC_CACHE is None:
        _NC_CACHE = build_nc()
    return _NC_CACHE


def _in_maps(inputs):
    x = np.ascontiguousarray(np.asarray(inputs["x"], dtype=np.float32).reshape(N, C))
    Wr = np.ascontiguousarray(np.asarray(inputs["Wr"], dtype=np.float32))
    br = np.ascontiguousarray(np.asarray(inputs["br"], dtype=np.float32))
    W1 = np.asarray(inputs["W1"], dtype=np.float32)
    b1 = np.asarray(inputs["b1"], dtype=np.float32)
    Wg = np.asarray(inputs["Wg"], dtype=np.float32)
    bg = np.asarray(inputs["bg"], dtype=np.float32)
    W2 = np.asarray(inputs["W2"], dtype=np.float32)
    b2 = np.asarray(inputs["b2"], dtype=np.float32)
    maps = []
    for c in range(N_CORES):
        esel = np.zeros((P, E), np.float32)
        esel[:, c] = 1.0
        maps.append({
            "x": x,
            "xsl": np.ascontiguousarray(x[c * SLICE:(c + 1) * SLICE]),
            "wr": Wr, "br": br, "esel": esel,
            "w1": np.ascontiguousarray(W1[c]),
            "b1": np.ascontiguousarray(b1[c]),
            "wg": np.ascontiguousarray(Wg[c]),
            "bg": np.ascontiguousarray(bg[c]),
            "w2": np.ascontiguousarray(W2[c]),
            "b2": np.ascontiguousarray(b2[c]),
        })
    return maps


def _assemble(results):
    # core c's y_slice = [half0 rows c*512:(c+1)*512, half1 rows ...]
    out = np.empty((N, C), np.float32)
    HS = HALF // N_CORES
    for c in range(N_CORES):
        ys = results[c]["y_slice"]
        out[c * HS:(c + 1) * HS] = ys[:HS]
        out[HALF + c * HS:HALF + (c + 1) * HS] = ys[HS:]
    return out


def _run(inputs, trace=False):
    from concourse.bass_utils import run_bass_kernel_spmd

    nc = _get_nc()
    res = run_bass_kernel_spmd(nc, _in_maps(inputs), list(range(N_CORES)), trace=trace)
    out = _assemble(res.results)
    return out.reshape(B, T, C), res


def kernel(**inputs) -> np.ndarray:
    out, _ = _run(inputs, trace=False)
    return out



# revision 2
# speedup vs baseline: 1.4318x; 1.4318x over previous
"""MoE top-2-of-8 SwiGLU feed-forward on 8 Trainium2 NeuronCores.

Strategy: expert-parallel, pipelined over two 4096-token halves, with an
AllToAll bucket combine (replaces the dense-partial ReduceScatter).
 - Router: core c routes tokens [c*1024,(c+1)*1024) in full fp32 on the PE
   (Wr-stationary: 8 LDWEIGHTS, tokens moving; top-2 selection must match the
   reference; smallest top2/top3 logit gap in this data is ~6e-5, far above
   fp32 matmul error). The combine-weight table w[N,8] is AllGathered.
 - Dispatch: core c builds the compacted slot list for its expert via
   prefix-sum matmuls. Slots are A2A-bucket-ordered: slot = owner*160 + rank
   within the (expert, owner, half) bucket, owner(t) = (t//512)%8. One
   indirect scatter per 128-token tile writes [w, token_id] into a DRAM side
   table; invalid tokens go OOB and are skipped (tails keep w=0/id=0).
 - FFN (per half): gather <=1280 tokens from a bf16 copy of x, transpose on
   the PE, h=x@W1+b1, g=x@Wg+bg, y=(silu(h)*g)@W2+b2, all bf16 with fp32
   PSUM accumulate, weight-stationary over [512,512,256] token blocks.
   Weights stay SBUF-resident in bf16 across halves. y rows are w-scaled and
   written straight into the A2A send buffer (slot order == buffer order).
 - Combine: AllToAll delivers bucket (e->o) at recv rows [e*160, e*160+cnt).
   Owner reconstructs its tokens' two bucket positions from the AllGathered
   w table (same prefix-sum ranks), gathers the two rows, adds, writes out.
   A2A(half0)+combine(half0) overlap FFN(half1).
"""
import numpy as np
import ml_dtypes

import concourse.bass as bass
import concourse.mybir as mybir
import concourse.tile as tile
from concourse.masks import make_identity
from concourse.vector_clock import ScopedClock

P = 128
N_CORES = 8
B, T, C, E = 4, 2048, 1024, 8
N = B * T                  # 8192 tokens
HALF = N // 2              # 4096 tokens per pipeline half
SLICE = N // N_CORES       # 1024 tokens per core router slice
NT_SL = SLICE // P         # 8 tiles per router slice
CC = C // P                # 8 feature chunks
BCAP = 160                 # rows per (expert, owner, half) A2A bucket
CAPF = BCAP * N_CORES      # 1280 = per-half compacted-token capacity
NST = CAPF // P            # 10 sub-tiles per half
BLOCKS = (512, 512, 256)   # FFN token blocks per half
BOFF = (0, 512, 1024)
BIG = 1.0e6
F32 = mybir.dt.float32
BF16 = mybir.dt.bfloat16
I32 = mybir.dt.int32
ACTF = mybir.ActivationFunctionType

# ---------------------------------------------------------------- tile patch
# Walrus in this environment accepts only ONE semaphore wait per instruction.
# Tile attaches several (end-of-kernel drain, multi-producer deps). Split the
# extras onto same-engine NoOps/Drains placed immediately before.


def _drain_and_barrier(self, tick_clock, wait_clock):
    drain_inst = self.nc.sync.drain()
    wait_clock.add_sem_waits(
        drain_inst.ins, ScopedClock({None: tick_clock.global_clock})
    )
    si = drain_inst.ins.sync_info
    if si is not None and si.on_wait is not None and len(si.on_wait) > 1:
        waits = list(si.on_wait)
        si.on_wait = waits[:1]
        for w in waits[1:]:
            extra = self.nc.sync.drain()
            esi = extra.ins.sync_info
            if esi is None:
                esi = mybir.SyncInfo(on_wait=[], on_update=[])
                extra.ins.sync_info = esi
            esi.on_wait = [w]
    self.nc.all_engine_barrier()
    assert self.sems is not None
    popped = self.nc._tile_sem_poison_stack.pop()
    assert popped is self._sem_poison
    self.nc.clear_and_free_semaphores(list(self.sems.allocated().values()))
    self.nc.all_engine_barrier()


tile.TileContext._drain_and_barrier = _drain_and_barrier


def split_multi_waits(nc, max_waits=1):
    for f in nc.m.functions:
        for bb in f.blocks:
            new = []
            dirty = False
            for ins in bb.instructions:
                si = getattr(ins, "sync_info", None)
                if si is not None and si.on_wait and len(si.on_wait) > max_waits:
                    waits = list(si.on_wait)
                    extra, keep = waits[:-max_waits], waits[-max_waits:]
                    for j in range(0, len(extra), max_waits):
                        nop = mybir.InstNoOp(
                            name=f"{ins.name}-wsplit{j}", ins=[], outs=[]
                        )
                        nop.engine = ins.engine
                        nop.sync_info = mybir.SyncInfo(
                            on_wait=extra[j : j + max_waits], on_update=[]
                        )
                        new.append(nop)
                    si.on_wait = keep
                    dirty = True
                new.append(ins)
            if dirty:
                bb.instructions = new


# ---------------------------------------------------------------- kernel IR


def build_nc():
    nc = bass.Bass()
    xbf_in = nc.declare_dram_parameter("xbf", [N, C], BF16, isOutput=False)
    xslT_in = nc.declare_dram_parameter("xslT", [C, SLICE], F32, isOutput=False)
    wr_in = nc.declare_dram_parameter("wr", [C, E], F32, isOutput=False)
    br_in = nc.declare_dram_parameter("br", [E], F32, isOutput=False)
    esel_in = nc.declare_dram_parameter("esel", [P, E], F32, isOutput=False)
    oidx_in = nc.declare_dram_parameter("ownidx", [P, 2], I32, isOutput=False)
    w1_in = nc.declare_dram_parameter("w1", [C, C], BF16, isOutput=False)
    b1_in = nc.declare_dram_parameter("b1", [C], F32, isOutput=False)
    wg_in = nc.declare_dram_parameter("wg", [C, C], BF16, isOutput=False)
    bg_in = nc.declare_dram_parameter("bg", [C], F32, isOutput=False)
    w2_in = nc.declare_dram_parameter("w2", [C, C], BF16, isOutput=False)
    b2_in = nc.declare_dram_parameter("b2", [C], F32, isOutput=False)
    y_out = nc.declare_dram_parameter("y_slice", [SLICE, C], F32, isOutput=True)

    w_sl = nc.dram_tensor("w_sl", [P, NT_SL * E], F32)  # row p, col tt*8+e
    w_all = nc.dram_tensor("w_all", [N_CORES * P, NT_SL * E], F32, addr_space="Shared")
    sidetabs = [nc.dram_tensor(f"sidetab{h}", [CAPF, 4], F32) for h in range(2)]
    a2a_send = [nc.dram_tensor(f"a2a_s{h}", [CAPF, C], BF16) for h in range(2)]
    a2a_recv = [nc.dram_tensor(f"a2a_r{h}", [CAPF, C], BF16) for h in range(2)]
    GROUPS = [list(range(N_CORES))]

    with tile.TileContext(nc) as tc:
        with tc.tile_pool(name="const", bufs=1) as cpool:
            breg_slot = nc.gpsimd.to_reg(CAPF - 1)
            ident = cpool.tile([P, P], F32)
            make_identity(nc, ident[:])
            identb = cpool.tile([P, P], BF16)
            nc.vector.tensor_copy(out=identb[:], in_=ident[:])
            ones1 = cpool.tile([1, 512], F32)
            nc.vector.memset(ones1[:], 1.0)
            ones128 = cpool.tile([P, P], F32)
            nc.vector.memset(ones128[:], 1.0)
            tri128 = cpool.tile([P, P], F32)
            nc.vector.memset(tri128[:], 1.0)
            nc.gpsimd.affine_select(
                out=tri128[:], in_=tri128[:], pattern=[[1, P]],
                compare_op=mybir.AluOpType.is_ge, fill=0.0,
                base=-1, channel_multiplier=-1)
            b1_sb = cpool.tile([P, CC], F32)
            nc.sync.dma_start(out=b1_sb[:], in_=b1_in.rearrange("(ic p) -> p ic", p=P))
            bg_sb = cpool.tile([P, CC], F32)
            nc.sync.dma_start(out=bg_sb[:], in_=bg_in.rearrange("(ic p) -> p ic", p=P))
            b2_sb = cpool.tile([P, CC], F32)
            nc.sync.dma_start(out=b2_sb[:], in_=b2_in.rearrange("(mc p) -> p mc", p=P))

            # resident bf16 expert weights: [p_c, cc, i] so lhsT chunk for
            # (contract cc, out ic) is w1sb[:, cc, ic*P:(ic+1)*P]
            wpool = tc.tile_pool(name="wres", bufs=1)
            wp = wpool.__enter__()
            w1sb = wp.tile([P, CC, C], BF16)
            nc.sync.dma_start(out=w1sb[:], in_=w1_in.rearrange("(cc p) i -> p cc i", p=P))
            wgsb = wp.tile([P, CC, C], BF16)
            nc.sync.dma_start(out=wgsb[:], in_=wg_in.rearrange("(cc p) i -> p cc i", p=P))
            w2sb = wp.tile([P, CC, C], BF16)
            nc.sync.dma_start(out=w2sb[:], in_=w2_in.rearrange("(ic p) c -> p ic c", p=P))

            # ---------------- phase R: router over this core's slice -------
            with (
                tc.tile_pool(name="rpool", bufs=1) as rp,
                tc.tile_pool(name="rpsum", bufs=1, space="PSUM") as rps,
            ):
                wr_sb = rp.tile([P, CC, E], F32, name="wr_sb")
                nc.sync.dma_start(out=wr_sb[:], in_=wr_in.rearrange("(cc p) e -> p cc e", p=P))
                br_sb = rp.tile([1, E], F32, name="br_sb")
                nc.sync.dma_start(out=br_sb[:], in_=br_in[None, :])
                xT_sb = rp.tile([P, CC, SLICE], F32, name="xT_sb")
                nc.sync.dma_start(out=xT_sb[:], in_=xslT_in.rearrange("(cc p) t -> p cc t", p=P))
                lgT = rp.tile([E, SLICE], F32, name="lgT")
                for b in range(2):
                    ps_l = rps.tile([E, 512], F32, name=f"psl{b}", tag="psl", bufs=2)
                    for cc in range(CC):
                        nc.tensor.matmul(out=ps_l[:], lhsT=wr_sb[:, cc],
                                         rhs=xT_sb[:, cc, b * 512:(b + 1) * 512],
                                         start=(cc == 0), stop=False)
                    nc.tensor.matmul(out=ps_l[:], lhsT=br_sb[:], rhs=ones1[:],
                                     start=False, stop=True)
                    nc.vector.tensor_copy(out=lgT[:, b * 512:(b + 1) * 512], in_=ps_l[:])
                lg_all = rp.tile([P, NT_SL, E], F32, name="lg_all")
                for tt in range(NT_SL):
                    ps_t = rps.tile([P, E], F32, name=f"rt{tt}", tag="pst", bufs=2)
                    nc.tensor.transpose(out=ps_t[:], in_=lgT[:, tt * P:(tt + 1) * P],
                                        identity=ident[0:E, 0:E])
                    nc.vector.tensor_copy(out=lg_all[:, tt], in_=ps_t[:])
                # batched softmax + top-2 over all 8 tiles
                s8_all = rp.tile([P, NT_SL, 8], F32, name="s8_all")
                for tt in range(NT_SL):
                    nc.vector.max(out=s8_all[:, tt], in_=lg_all[:, tt])
                lsh = rp.tile([P, NT_SL, E], F32, name="lsh")
                nc.vector.tensor_tensor(out=lsh[:], in0=lg_all[:],
                                        in1=s8_all[:, :, 0:1].to_broadcast([P, NT_SL, E]),
                                        op=mybir.AluOpType.subtract)
                ex_all = rp.tile([P, NT_SL, E], F32, name="ex_all")
                nc.scalar.activation(ex_all[:], lsh[:], ACTF.Exp)
                ssum = rp.tile([P, NT_SL], F32, name="ssum")
                nc.vector.reduce_sum(out=ssum[:], in_=ex_all[:], axis=mybir.AxisListType.X)
                rec = rp.tile([P, NT_SL], F32, name="rec")
                nc.vector.reciprocal(rec[:], ssum[:])
                mk = rp.tile([P, NT_SL, E], F32, name="mk")
                nc.vector.tensor_tensor(out=mk[:], in0=lg_all[:],
                                        in1=s8_all[:, :, 1:2].to_broadcast([P, NT_SL, E]),
                                        op=mybir.AluOpType.is_ge)
                wt_all = rp.tile([P, NT_SL, E], F32, name="wt_all")
                nc.vector.tensor_tensor(out=wt_all[:], in0=ex_all[:],
                                        in1=rec[:].unsqueeze(2).to_broadcast([P, NT_SL, E]),
                                        op=mybir.AluOpType.mult)
                nc.vector.tensor_mul(wt_all[:], wt_all[:], mk[:])
                nc.sync.dma_start(out=w_sl.rearrange("p (tt e) -> p tt e", e=E),
                                  in_=wt_all[:])

            # side-table templates (w=0, id=0 defaults for bucket tails),
            # on the scalar DMA queue after the router's Exp activation
            tmpl = cpool.tile([P, NST, 4], F32)
            nc.vector.memset(tmpl[:], 0.0)
            for h in range(2):
                nc.scalar.dma_start(
                    out=sidetabs[h].rearrange("(st p) c -> p st c", p=P), in_=tmpl[:])
            nc.gpsimd.collective_compute(
                "AllGather", mybir.AluOpType.bypass, replica_groups=GROUPS,
                ins=[w_sl[:]], outs=[w_all[:]],
            )

            # ---------------- phase D: dispatch + owner ranks --------------
            # cpp holds tiles the combine phase reads much later
            cpp = tc.tile_pool(name="cpers", bufs=1)
            cp = cpp.__enter__()
            i1 = [None, None]
            i2 = [None, None]
            with (
                tc.tile_pool(name="dpool", bufs=1) as dpool,
                tc.tile_pool(name="dpsum", bufs=1, space="PSUM") as dps,
            ):
                esel = dpool.tile([P, E], F32)
                nc.sync.dma_start(out=esel[:], in_=esel_in[:])
                oidx = dpool.tile([P, 2], I32)
                nc.sync.dma_start(out=oidx[:], in_=oidx_in[:])
                NG = 2 * 32  # tiles across both halves
                w8 = dpool.tile([P, NG, E], F32, name="w8")
                nc.sync.dma_start(
                    out=w8.rearrange("p (c tt) e -> p c tt e", tt=NT_SL),
                    in_=w_all.rearrange("(c p) (tt e) -> p c tt e", p=P, e=E))
                wcol = dpool.tile([P, NG], F32, name="wc")
                wprod = dpool.tile([P, NG, E], F32, name="wp")
                nc.vector.tensor_mul(
                    wprod[:], w8[:], esel[:].unsqueeze(1).to_broadcast([P, NG, E]))
                nc.vector.reduce_sum(out=wcol[:], in_=wprod[:], axis=mybir.AxisListType.X)
                mask = dpool.tile([P, NG], F32, name="dm")
                nc.vector.tensor_scalar(mask[:], wcol[:], 0.0, scalar2=None,
                                        op0=mybir.AluOpType.is_gt)
                # cross-tile shifted masks within each owner group of 4 tiles
                msk = dpool.tile([P, NG], F32, name="ms")
                mv = msk.rearrange("p (g j) -> p g j", j=4)
                m4v = mask.rearrange("p (g j) -> p g j", j=4)
                nc.vector.memset(mv[:, :, 0], 0.0)
                nc.vector.tensor_copy(out=mv[:, :, 1], in_=m4v[:, :, 0])
                nc.vector.tensor_add(mv[:, :, 2], mv[:, :, 1], m4v[:, :, 1])
                nc.vector.tensor_add(mv[:, :, 3], mv[:, :, 2], m4v[:, :, 2])
                ps_rank = dps.tile([P, NG], F32, name="psrank", tag="psr")
                nc.tensor.matmul(out=ps_rank[:], lhsT=tri128[:], rhs=mask[:],
                                 start=True, stop=False)
                nc.tensor.matmul(out=ps_rank[:], lhsT=ones128[:], rhs=msk[:],
                                 start=False, stop=True)
                rank = dpool.tile([P, NG], F32, name="rank")
                nc.vector.tensor_copy(out=rank[:], in_=ps_rank[:])
                obase_i = dpool.tile([P, NG], I32, name="obi")
                nc.gpsimd.iota(obase_i.rearrange("p (h o j) -> p h o j", o=8, j=4),
                               pattern=[[0, 2], [BCAP, 8], [0, 4]],
                               base=0, channel_multiplier=0)
                obase_f = dpool.tile([P, NG], F32, name="obf")
                nc.vector.tensor_copy(out=obase_f[:], in_=obase_i[:])
                slotf = dpool.tile([P, NG], F32, name="sf")
                nc.vector.tensor_add(slotf[:], rank[:], obase_f[:])
                nc.vector.tensor_mul(slotf[:], slotf[:], mask[:])
                m1m = dpool.tile([P, NG], F32, name="m1m")
                nc.vector.tensor_scalar(m1m[:], mask[:], -1.0, scalar2=1.0,
                                        op0=mybir.AluOpType.mult, op1=mybir.AluOpType.add)
                nc.vector.tensor_scalar_mul(m1m[:], m1m[:], BIG)
                nc.vector.tensor_add(slotf[:], slotf[:], m1m[:])
                slot_i = dpool.tile([P, NG], I32, name="si")
                nc.vector.tensor_copy(out=slot_i[:], in_=slotf[:])
                gid_i = dpool.tile([P, NG], I32, name="gi")
                nc.gpsimd.iota(gid_i[:], pattern=[[P, NG]], base=0, channel_multiplier=1)
                side = dpool.tile([P, NG, 4], F32, name="side")
                nc.vector.memset(side[:], 0.0)
                nc.vector.tensor_copy(out=side[:, :, 0], in_=wcol[:])
                nc.vector.tensor_copy(out=side[:, :, 1], in_=gid_i[:])
                for g in range(NG):
                    h = g // 32
                    st_ap = sidetabs[h][0:1, :]
                    st_ap = bass.AP(tensor=st_ap.tensor, offset=0, ap=st_ap.ap,
                                    dep_tracking_offset=(g % 32) * 4)
                    nc.gpsimd.indirect_dma_start(
                        out=st_ap,
                        out_offset=bass.IndirectOffsetOnAxis(ap=slot_i[:, g:g + 1], axis=0),
                        in_=side[:, g, :], in_offset=None,
                        bounds_check=breg_slot, oob_is_err=False,
                    )
                # owner-side bucket positions for this core's output tokens
                ebase_i = dpool.tile([P, E], I32, name="ebi")
                nc.gpsimd.iota(ebase_i[:], pattern=[[BCAP, E]], base=0,
                               channel_multiplier=0)
                ebase_f = dpool.tile([P, E], F32, name="ebf")
                nc.vector.tensor_copy(out=ebase_f[:], in_=ebase_i[:])
                w_all2 = w_all.rearrange("r (b f) -> (r b) f", b=2)
                for h in range(2):
                    w4 = dpool.tile([P, 32], F32, name=f"w4_{h}", tag="w4")
                    nc.gpsimd.indirect_dma_start(
                        out=w4[:], out_offset=None,
                        in_=w_all2,
                        in_offset=bass.IndirectOffsetOnAxis(ap=oidx[:, h:h + 1], axis=0),
                    )
                    m4 = dpool.tile([P, 32], F32, name=f"m4_{h}", tag="m4")
                    nc.vector.tensor_scalar(m4[:], w4[:], 0.0, scalar2=None,
                                            op0=mybir.AluOpType.is_gt)
                    ms4 = dpool.tile([P, 32], F32, name=f"ms4_{h}", tag="ms4")
                    msv = ms4.rearrange("p (j e) -> p j e", e=E)
                    m4j = m4.rearrange("p (j e) -> p j e", e=E)
                    nc.vector.memset(msv[:, 0], 0.0)
                    nc.vector.tensor_copy(out=msv[:, 1], in_=m4j[:, 0])
                    nc.vector.tensor_add(msv[:, 2], msv[:, 1], m4j[:, 1])
                    nc.vector.tensor_add(msv[:, 3], msv[:, 2], m4j[:, 2])
                    ps_r4 = dps.tile([P, 32], F32, name=f"psr4_{h}", tag="psr4")
                    nc.tensor.matmul(out=ps_r4[:], lhsT=tri128[:], rhs=m4[:],
                                     start=True, stop=False)
                    nc.tensor.matmul(out=ps_r4[:], lhsT=ones128[:], rhs=ms4[:],
                                     start=False, stop=True)
                    pos = dpool.tile([P, 4, E], F32, name=f"pos_{h}", tag="pos")
                    nc.vector.tensor_copy(out=pos[:], in_=ps_r4.rearrange("p (j e) -> p j e", e=E))
                    nc.vector.tensor_tensor(
                        out=pos[:], in0=pos[:],
                        in1=ebase_f[:].unsqueeze(1).to_broadcast([P, 4, E]),
                        op=mybir.AluOpType.add)
                    nc.vector.tensor_mul(pos[:], pos[:], m4j[:])
                    p2 = dpool.tile([P, 4], F32, name=f"p2_{h}", tag="p2")
                    nc.vector.reduce_max(out=p2[:], in_=pos[:], axis=mybir.AxisListType.X)
                    p1 = dpool.tile([P, 4], F32, name=f"p1_{h}", tag="p1")
                    nc.vector.reduce_sum(out=p1[:], in_=pos[:], axis=mybir.AxisListType.X)
                    nc.vector.tensor_tensor(out=p1[:], in0=p1[:], in1=p2[:],
                                            op=mybir.AluOpType.subtract)
                    i1[h] = cp.tile([P, 4], I32, name=f"i1_{h}")
                    nc.vector.tensor_copy(out=i1[h][:], in_=p1[:])
                    i2[h] = cp.tile([P, 4], I32, name=f"i2_{h}")
                    nc.vector.tensor_copy(out=i2[h][:], in_=p2[:])

            # ---------------- phase F: expert FFN + A2A + combine ----------
            with (
                tc.tile_pool(name="fbig", bufs=2) as fbig,
                tc.tile_pool(name="fa", bufs=1) as fa,
                tc.tile_pool(name="fxg", bufs=6) as fxg,
                tc.tile_pool(name="fsmall", bufs=2) as fs,
                tc.tile_pool(name="fpsum", bufs=1, space="PSUM") as fps,
            ):
                for h in range(2):
                    sv = fs.tile([P, NST, 4], F32, name=f"sv{h}", tag="sv")
                    nc.sync.dma_start(out=sv[:], in_=sidetabs[h].rearrange("(st p) c -> p st c", p=P))
                    wv = fs.tile([P, NST], F32, name=f"wv{h}", tag="wv")
                    nc.vector.tensor_copy(out=wv[:], in_=sv[:, :, 0])
                    idg = fs.tile([P, NST], I32, name=f"idg{h}", tag="idg")
                    nc.vector.tensor_copy(out=idg[:], in_=sv[:, :, 1])
                    xgT = fbig.tile([P, CC, CAPF], BF16, name=f"xgT{h}", tag="big")
                    for st in range(NST):
                        xg = fxg.tile([P, C], BF16, name=f"xg_{h}_{st}", tag="xg")
                        nc.gpsimd.indirect_dma_start(
                            out=xg[:], out_offset=None,
                            in_=xbf_in[:],
                            in_offset=bass.IndirectOffsetOnAxis(ap=idg[:, st:st + 1], axis=0),
                        )
                        for cc in range(CC):
                            ps_t = fps.tile([P, P], BF16, name=f"ft{h}_{st}_{cc}",
                                            tag="trA" if (st * CC + cc) % 2 == 0 else "trB")
                            nc.tensor.transpose(out=ps_t[:], in_=xg[:, cc * P:(cc + 1) * P],
                                                identity=identb[:])
                            nc.vector.tensor_copy(out=xgT[:, cc, st * P:(st + 1) * P],
                                                  in_=ps_t[:])
                    # L1
                    a_t = fa.tile([P, CC, CAPF], BF16, name=f"a{h}", tag="abuf")
                    for ic in range(CC):
                        ps_h = [fps.tile([P, 512], F32, name=f"psh{h}_{ic}_{b}",
                                         tag=f"mmA{b}") for b in range(len(BLOCKS))]
                        for cc in range(CC):
                            for b, bw in enumerate(BLOCKS):
                                nc.tensor.matmul(out=ps_h[b][:, :bw],
                                                 lhsT=w1sb[:, cc, ic * P:(ic + 1) * P],
                                                 rhs=xgT[:, cc, BOFF[b]:BOFF[b] + bw],
                                                 start=(cc == 0), stop=(cc == CC - 1))
                        sil = fs.tile([P, CAPF], BF16, name=f"sil{h}_{ic}", tag="sil")
                        for b, bw in enumerate(BLOCKS):
                            nc.scalar.activation(sil[:, BOFF[b]:BOFF[b] + bw], ps_h[b][:, :bw],
                                                 ACTF.Silu, bias=b1_sb[:, ic:ic + 1])
                        ps_g = [fps.tile([P, 512], F32, name=f"psg{h}_{ic}_{b}",
                                         tag=f"mmB{b}") for b in range(len(BLOCKS))]
                        for cc in range(CC):
                            for b, bw in enumerate(BLOCKS):
                                nc.tensor.matmul(out=ps_g[b][:, :bw],
                                                 lhsT=wgsb[:, cc, ic * P:(ic + 1) * P],
                                                 rhs=xgT[:, cc, BOFF[b]:BOFF[b] + bw],
                                                 start=(cc == 0), stop=(cc == CC - 1))
                        g_sb = fs.tile([P, CAPF], BF16, name=f"g{h}_{ic}", tag="gsb")
                        for b, bw in enumerate(BLOCKS):
                            nc.scalar.activation(g_sb[:, BOFF[b]:BOFF[b] + bw], ps_g[b][:, :bw],
                                                 ACTF.Identity, bias=bg_sb[:, ic:ic + 1])
                        nc.vector.tensor_mul(a_t[:, ic], sil[:], g_sb[:])

                    # L2
                    y_tok = fbig.tile([P, NST, C], BF16, name=f"ytok{h}", tag="big")
                    for mc in range(CC):
                        ps_y = [fps.tile([P, 512], F32, name=f"psy{h}_{mc}_{b}",
                                         tag=f"mm{'A' if mc % 2 == 0 else 'B'}{b}")
                                for b in range(len(BLOCKS))]
                        for ic in range(CC):
                            for b, bw in enumerate(BLOCKS):
                                nc.tensor.matmul(out=ps_y[b][:, :bw],
                                                 lhsT=w2sb[:, ic, mc * P:(mc + 1) * P],
                                                 rhs=a_t[:, ic, BOFF[b]:BOFF[b] + bw],
                                                 start=(ic == 0), stop=(ic == CC - 1))
                        y_sb = fs.tile([P, CAPF], BF16, name=f"ysb{h}_{mc}", tag="ysb")
                        for b, bw in enumerate(BLOCKS):
                            nc.scalar.activation(y_sb[:, BOFF[b]:BOFF[b] + bw], ps_y[b][:, :bw],
                                                 ACTF.Identity, bias=b2_sb[:, mc:mc + 1])
                        for st in range(NST):
                            ps_t2 = fps.tile([P, P], BF16, name=f"bt{h}_{mc}_{st}",
                                             tag="trA" if (st + mc) % 2 == 0 else "trB")
                            nc.tensor.transpose(out=ps_t2[:],
                                                in_=y_sb[:, st * P:(st + 1) * P],
                                                identity=identb[:])
                            nc.vector.tensor_copy(out=y_tok[:, st, mc * P:(mc + 1) * P],
                                                  in_=ps_t2[:])
                    for st in range(NST):
                        nc.vector.tensor_scalar_mul(y_tok[:, st], y_tok[:, st],
                                                    wv[:, st:st + 1])
                        nc.sync.dma_start(out=a2a_send[h][st * P:(st + 1) * P, :],
                                          in_=y_tok[:, st])
                    nc.gpsimd.collective_compute(
                        "AllToAll", mybir.AluOpType.bypass, replica_groups=GROUPS,
                        ins=[a2a_send[h][:]], outs=[a2a_recv[h][:]],
                    )
                    # combine this half while the next half computes
                    for j in range(4):
                        g1 = fs.tile([P, C], BF16, name=f"cg1_{h}_{j}", tag="cg", bufs=4)
                        nc.gpsimd.indirect_dma_start(
                            out=g1[:], out_offset=None,
                            in_=a2a_recv[h][:],
                            in_offset=bass.IndirectOffsetOnAxis(ap=i1[h][:, j:j + 1], axis=0),
                        )
                        g2 = fs.tile([P, C], BF16, name=f"cg2_{h}_{j}", tag="cg", bufs=4)
                        nc.gpsimd.indirect_dma_start(
                            out=g2[:], out_offset=None,
                            in_=a2a_recv[h][:],
                            in_offset=bass.IndirectOffsetOnAxis(ap=i2[h][:, j:j + 1], axis=0),
                        )
                        ot = fs.tile([P, C], F32, name=f"ot_{h}_{j}", tag="ot")
                        nc.vector.tensor_tensor(out=ot[:], in0=g1[:], in1=g2[:],
                                                op=mybir.AluOpType.add)
                        nc.sync.dma_start(
                            out=y_out[h * 512 + j * P:h * 512 + (j + 1) * P, :], in_=ot[:])
            cpp.__exit__(None, None, None)
            wpool.__exit__(None, None, None)

    split_multi_waits(nc)
    return nc


_NC_CACHE = None


def _get_nc():
    global _NC_CACHE
    if _NC_CACHE is None:
        _NC_CACHE = build_nc()
    return _NC_CACHE


def _in_maps(inputs):
    bf16 = ml_dtypes.bfloat16
    x = np.ascontiguousarray(np.asarray(inputs["x"], dtype=np.float32).reshape(N, C))
    xbf = np.ascontiguousarray(x.astype(bf16))
    Wr = np.ascontiguousarray(np.asarray(inputs["Wr"], dtype=np.float32))
    br = np.ascontiguousarray(np.asarray(inputs["br"], dtype=np.float32))
    W1 = np.asarray(inputs["W1"], dtype=np.float32)
    b1 = np.asarray(inputs["b1"], dtype=np.float32)
    Wg = np.asarray(inputs["Wg"], dtype=np.float32)
    bg = np.asarray(inputs["bg"], dtype=np.float32)
    W2 = np.asarray(inputs["W2"], dtype=np.float32)
    b2 = np.asarray(inputs["b2"], dtype=np.float32)
    maps = []
    for c in range(N_CORES):
        esel = np.zeros((P, E), np.float32)
        esel[:, c] = 1.0
        ownidx = np.zeros((P, 2), np.int32)
        for h in range(2):
            r = 4 * h + c // 2
            ownidx[:, h] = (r * P + np.arange(P)) * 2 + (c % 2)
        maps.append({
            "xbf": xbf,
            "xslT": np.ascontiguousarray(x[c * SLICE:(c + 1) * SLICE].T),
            "wr": Wr, "br": br, "esel": esel, "ownidx": ownidx,
            "w1": np.ascontiguousarray(W1[c].astype(bf16)),
            "b1": np.ascontiguousarray(b1[c]),
            "wg": np.ascontiguousarray(Wg[c].astype(bf16)),
            "bg": np.ascontiguousarray(bg[c]),
            "w2": np.ascontiguousarray(W2[c].astype(bf16)),
            "b2": np.ascontiguousarray(b2[c]),
        })
    return maps


def _assemble(results):
    # core c's y_slice = [half0 rows c*512:(c+1)*512, half1 rows ...]
    out = np.empty((N, C), np.float32)
    HS = 512
    for c in range(N_CORES):
        ys = results[c]["y_slice"]
        out[c * HS:(c + 1) * HS] = ys[:HS]
        out[HALF + c * HS:HALF + (c + 1) * HS] = ys[HS:]
    return out


def _run(inputs, trace=False):
    from concourse.bass_utils import run_bass_kernel_spmd

    nc = _get_nc()
    res = run_bass_kernel_spmd(nc, _in_maps(inputs), list(range(N_CORES)), trace=trace)
    out = _assemble(res.results)
    return out.reshape(B, T, C), res


def kernel(**inputs) -> np.ndarray:
    out, _ = _run(inputs, trace=False)
    return out


# revision 15
# speedup vs baseline: 1.7000x; 1.1873x over previous
"""MoE top-2-of-8 SwiGLU feed-forward on 8 Trainium2 NeuronCores.

Strategy: expert-parallel, pipelined over two 4096-token halves, with an
AllToAll bucket combine (replaces the dense-partial ReduceScatter).
 - Router: core c routes tokens [c*1024,(c+1)*1024) in full fp32 on the PE
   (Wr-stationary: 8 LDWEIGHTS, tokens moving; top-2 selection must match the
   reference; smallest top2/top3 logit gap in this data is ~6e-5, far above
   fp32 matmul error). The combine-weight table w[N,8] is AllGathered.
 - Dispatch: core c builds the compacted slot list for its expert via
   prefix-sum matmuls. Slots are A2A-bucket-ordered: slot = owner*160 + rank
   within the (expert, owner, half) bucket, owner(t) = (t//512)%8. One
   indirect scatter per 128-token tile writes [w, token_id] into a DRAM side
   table; invalid tokens go OOB and are skipped (tails keep w=0/id=0).
 - FFN (per half): gather <=1280 tokens from a bf16 copy of x, transpose on
   the PE, h=x@W1+b1, g=x@Wg+bg, y=(silu(h)*g)@W2+b2, all bf16 with fp32
   PSUM accumulate, weight-stationary over [512,512,256] token blocks.
   Weights stay SBUF-resident in bf16 across halves. y rows are w-scaled and
   written straight into the A2A send buffer (slot order == buffer order).
 - Combine: AllToAll delivers bucket (e->o) at recv rows [e*160, e*160+cnt).
   Owner reconstructs its tokens' two bucket positions from the AllGathered
   w table (same prefix-sum ranks), gathers the two rows, adds, writes out.
   A2A(half0)+combine(half0) overlap FFN(half1).
"""
import numpy as np
import ml_dtypes

import concourse.bass as bass
import concourse.mybir as mybir
import concourse.tile as tile
from concourse.masks import make_identity
from concourse.vector_clock import ScopedClock

P = 128
N_CORES = 8
B, T, C, E = 4, 2048, 1024, 8
N = B * T                  # 8192 tokens
HALF = N // 2              # 4096 tokens per pipeline half
SLICE = N // N_CORES       # 1024 tokens per core router slice
NT_SL = SLICE // P         # 8 tiles per router slice
CC = C // P                # 8 feature chunks
BCAP = 160                 # rows per (expert, owner, half) A2A bucket
CAPF = BCAP * N_CORES      # 1280 = per-half compacted-token capacity
NST = CAPF // P            # 10 sub-tiles per half
BLOCKS = (512, 512, 256)   # FFN token blocks per half
BOFF = (0, 512, 1024)
BIG = 1.0e6
F32 = mybir.dt.float32
BF16 = mybir.dt.bfloat16
I32 = mybir.dt.int32
ACTF = mybir.ActivationFunctionType

# ---------------------------------------------------------------- tile patch
# Walrus in this environment accepts only ONE semaphore wait per instruction.
# Tile attaches several (end-of-kernel drain, multi-producer deps). Split the
# extras onto same-engine NoOps/Drains placed immediately before.


def _drain_and_barrier(self, tick_clock, wait_clock):
    drain_inst = self.nc.sync.drain()
    wait_clock.add_sem_waits(
        drain_inst.ins, ScopedClock({None: tick_clock.global_clock})
    )
    si = drain_inst.ins.sync_info
    if si is not None and si.on_wait is not None and len(si.on_wait) > 1:
        waits = list(si.on_wait)
        si.on_wait = waits[:1]
        for w in waits[1:]:
            extra = self.nc.sync.drain()
            esi = extra.ins.sync_info
            if esi is None:
                esi = mybir.SyncInfo(on_wait=[], on_update=[])
                extra.ins.sync_info = esi
            esi.on_wait = [w]
    self.nc.all_engine_barrier()
    assert self.sems is not None
    popped = self.nc._tile_sem_poison_stack.pop()
    assert popped is self._sem_poison
    self.nc.clear_and_free_semaphores(list(self.sems.allocated().values()))
    self.nc.all_engine_barrier()


tile.TileContext._drain_and_barrier = _drain_and_barrier


def split_multi_waits(nc, max_waits=1):
    for f in nc.m.functions:
        for bb in f.blocks:
            new = []
            dirty = False
            for ins in bb.instructions:
                si = getattr(ins, "sync_info", None)
                if si is not None and si.on_wait and len(si.on_wait) > max_waits:
                    waits = list(si.on_wait)
                    extra, keep = waits[:-max_waits], waits[-max_waits:]
                    for j in range(0, len(extra), max_waits):
                        nop = mybir.InstNoOp(
                            name=f"{ins.name}-wsplit{j}", ins=[], outs=[]
                        )
                        nop.engine = ins.engine
                        nop.sync_info = mybir.SyncInfo(
                            on_wait=extra[j : j + max_waits], on_update=[]
                        )
                        new.append(nop)
                    si.on_wait = keep
                    dirty = True
                new.append(ins)
            if dirty:
                bb.instructions = new


# ---------------------------------------------------------------- kernel IR


def build_nc():
    nc = bass.Bass()
    xbf_in = nc.declare_dram_parameter("xbf", [N, C], BF16, isOutput=False)
    xslT_in = nc.declare_dram_parameter("xslT", [C, SLICE], F32, isOutput=False)
    wr_in = nc.declare_dram_parameter("wr", [C, E], F32, isOutput=False)
    br_in = nc.declare_dram_parameter("br", [E], F32, isOutput=False)
    gid_in = nc.declare_dram_parameter("gidsl", [P, NT_SL], F32, isOutput=False)
    oidx_in = nc.declare_dram_parameter("ownidx", [P, 2], I32, isOutput=False)
    w1_in = nc.declare_dram_parameter("w1", [C, C], BF16, isOutput=False)
    b1_in = nc.declare_dram_parameter("b1", [C], F32, isOutput=False)
    wg_in = nc.declare_dram_parameter("wg", [C, C], BF16, isOutput=False)
    bg_in = nc.declare_dram_parameter("bg", [C], F32, isOutput=False)
    w2_in = nc.declare_dram_parameter("w2", [C, C], BF16, isOutput=False)
    b2_in = nc.declare_dram_parameter("b2", [C], F32, isOutput=False)
    y_out = nc.declare_dram_parameter("y_slice", [SLICE, C], F32, isOutput=True)

    w_sl = nc.dram_tensor("w_sl", [P, NT_SL * E], F32)  # row p, col tt*8+e
    w_all = nc.dram_tensor("w_all", [N_CORES * P, NT_SL * E], F32, addr_space="Shared")
    # dispatch side-table A2A: core r sends, for each expert e, a [2*BCAP, 2]
    # block of (w, token_id) rows at bucket positions; the A2A concatenation
    # by source IS the expert's slot-ordered side table (both halves).
    dspA_s = nc.dram_tensor("dspA_s", [2 * CAPF, 2], F32)
    dspA_r = nc.dram_tensor("dspA_r", [2 * CAPF, 2], F32)
    a2a_send = [nc.dram_tensor(f"a2a_s{h}", [CAPF, C], BF16) for h in range(2)]
    a2a_recv = [nc.dram_tensor(f"a2a_r{h}", [CAPF, C], BF16) for h in range(2)]
    GROUPS = [list(range(N_CORES))]

    with tile.TileContext(nc) as tc:
        with tc.tile_pool(name="const", bufs=1) as cpool:
            ident = cpool.tile([P, P], F32)
            make_identity(nc, ident[:])
            identb = cpool.tile([P, P], BF16)
            nc.vector.tensor_copy(out=identb[:], in_=ident[:])
            ones1 = cpool.tile([1, 512], F32)
            nc.vector.memset(ones1[:], 1.0)
            ones128 = cpool.tile([P, P], F32)
            nc.vector.memset(ones128[:], 1.0)
            tri128 = cpool.tile([P, P], F32)
            nc.vector.memset(tri128[:], 1.0)
            nc.gpsimd.affine_select(
                out=tri128[:], in_=tri128[:], pattern=[[1, P]],
                compare_op=mybir.AluOpType.is_ge, fill=0.0,
                base=-1, channel_multiplier=-1)
            b1_sb = cpool.tile([P, CC], F32)
            nc.sync.dma_start(out=b1_sb[:], in_=b1_in.rearrange("(ic p) -> p ic", p=P))
            bg_sb = cpool.tile([P, CC], F32)
            nc.sync.dma_start(out=bg_sb[:], in_=bg_in.rearrange("(ic p) -> p ic", p=P))
            b2_sb = cpool.tile([P, CC], F32)
            nc.sync.dma_start(out=b2_sb[:], in_=b2_in.rearrange("(mc p) -> p mc", p=P))

            # resident bf16 expert weights: [p_c, cc, i] so lhsT chunk for
            # (contract cc, out ic) is w1sb[:, cc, ic*P:(ic+1)*P].
            # Tiles allocated here; their DMAs are emitted after the router's
            # input loads so the router is not queued behind 6MB of weights.
            wpool = tc.tile_pool(name="wres", bufs=1)
            wp = wpool.__enter__()
            w1sb = wp.tile([P, CC, C], BF16)
            wgsb = wp.tile([P, CC, C], BF16)
            w2sb = wp.tile([P, CC, C], BF16)

            # ---------------- phase R: router over this core's slice -------
            with (
                tc.tile_pool(name="rpool", bufs=1) as rp,
                tc.tile_pool(name="rpsum", bufs=1, space="PSUM") as rps,
            ):
                wr_sb = rp.tile([P, CC, E], F32, name="wr_sb")
                nc.sync.dma_start(out=wr_sb[:], in_=wr_in.rearrange("(cc p) e -> p cc e", p=P))
                br_sb = rp.tile([1, E], F32, name="br_sb")
                nc.sync.dma_start(out=br_sb[:], in_=br_in[None, :])
                xT_sb = rp.tile([P, CC, SLICE], F32, name="xT_sb")
                nc.sync.dma_start(out=xT_sb[:], in_=xslT_in.rearrange("(cc p) t -> p cc t", p=P))
                nc.sync.dma_start(out=w1sb[:], in_=w1_in.rearrange("(cc p) i -> p cc i", p=P))
                nc.sync.dma_start(out=wgsb[:], in_=wg_in.rearrange("(cc p) i -> p cc i", p=P))
                nc.sync.dma_start(out=w2sb[:], in_=w2_in.rearrange("(ic p) c -> p ic c", p=P))
                lgT = rp.tile([E, SLICE], F32, name="lgT")
                for b in range(2):
                    ps_l = rps.tile([E, 512], F32, name=f"psl{b}", tag="psl", bufs=2)
                    for cc in range(CC):
                        nc.tensor.matmul(out=ps_l[:], lhsT=wr_sb[:, cc],
                                         rhs=xT_sb[:, cc, b * 512:(b + 1) * 512],
                                         start=(cc == 0), stop=False)
                    nc.tensor.matmul(out=ps_l[:], lhsT=br_sb[:], rhs=ones1[:],
                                     start=False, stop=True)
                    nc.vector.tensor_copy(out=lgT[:, b * 512:(b + 1) * 512], in_=ps_l[:])
                lg_all = rp.tile([P, NT_SL, E], F32, name="lg_all")
                for tt in range(NT_SL):
                    ps_t = rps.tile([P, E], F32, name=f"rt{tt}", tag="pst", bufs=2)
                    nc.tensor.transpose(out=ps_t[:], in_=lgT[:, tt * P:(tt + 1) * P],
                                        identity=ident[0:E, 0:E])
                    nc.vector.tensor_copy(out=lg_all[:, tt], in_=ps_t[:])
                # batched softmax + top-2 over all 8 tiles
                s8_all = rp.tile([P, NT_SL, 8], F32, name="s8_all")
                for tt in range(NT_SL):
                    nc.vector.max(out=s8_all[:, tt], in_=lg_all[:, tt])
                lsh = rp.tile([P, NT_SL, E], F32, name="lsh")
                nc.vector.tensor_tensor(out=lsh[:], in0=lg_all[:],
                                        in1=s8_all[:, :, 0:1].to_broadcast([P, NT_SL, E]),
                                        op=mybir.AluOpType.subtract)
                ex_all = rp.tile([P, NT_SL, E], F32, name="ex_all")
                nc.scalar.activation(ex_all[:], lsh[:], ACTF.Exp)
                ssum = rp.tile([P, NT_SL], F32, name="ssum")
                nc.vector.reduce_sum(out=ssum[:], in_=ex_all[:], axis=mybir.AxisListType.X)
                rec = rp.tile([P, NT_SL], F32, name="rec")
                nc.vector.reciprocal(rec[:], ssum[:])
                mk = rp.tile([P, NT_SL, E], F32, name="mk")
                nc.vector.tensor_tensor(out=mk[:], in0=lg_all[:],
                                        in1=s8_all[:, :, 1:2].to_broadcast([P, NT_SL, E]),
                                        op=mybir.AluOpType.is_ge)
                wt_all = rp.tile([P, NT_SL, E], F32, name="wt_all")
                nc.vector.tensor_tensor(out=wt_all[:], in0=ex_all[:],
                                        in1=rec[:].unsqueeze(2).to_broadcast([P, NT_SL, E]),
                                        op=mybir.AluOpType.mult)
                nc.vector.tensor_mul(wt_all[:], wt_all[:], mk[:])
                nc.sync.dma_start(out=w_sl.rearrange("p (tt e) -> p tt e", e=E),
                                  in_=wt_all[:])

                # -------- phase D: dispatch scatter (local slice only) -----
                # prefill the dispatch A2A send buffer (w=0, id=0 tails)
                tmpl = cpool.tile([P, 2 * NST, 2], F32)
                nc.vector.memset(tmpl[:], 0.0)
                nc.scalar.dma_start(
                    out=dspA_s.rearrange("(st p) c -> p st c", p=P), in_=tmpl[:])
                gid_sb = rp.tile([P, NT_SL], F32, name="gid_sb")
                nc.sync.dma_start(out=gid_sb[:], in_=gid_in[:])
                m = rp.tile([P, NT_SL, E], F32, name="m")
                nc.vector.tensor_scalar(m[:], wt_all[:], 0.0, scalar2=None,
                                        op0=mybir.AluOpType.is_gt)
                # cross-tile shifted masks within each owner group of 4 tiles
                msk = rp.tile([P, NT_SL, E], F32, name="msk")
                mv = msk.rearrange("p (g j) e -> p g j e", j=4)
                mjv = m.rearrange("p (g j) e -> p g j e", j=4)
                nc.vector.memset(mv[:, :, 0], 0.0)
                nc.vector.tensor_copy(out=mv[:, :, 1], in_=mjv[:, :, 0])
                nc.vector.tensor_add(mv[:, :, 2], mv[:, :, 1], mjv[:, :, 1])
                nc.vector.tensor_add(mv[:, :, 3], mv[:, :, 2], mjv[:, :, 2])
                ps_rank = rps.tile([P, NT_SL * E], F32, name="psrank", tag="psr")
                nc.tensor.matmul(out=ps_rank[:], lhsT=tri128[:],
                                 rhs=m.rearrange("p tt e -> p (tt e)"),
                                 start=True, stop=False)
                nc.tensor.matmul(out=ps_rank[:], lhsT=ones128[:],
                                 rhs=msk.rearrange("p tt e -> p (tt e)"),
                                 start=False, stop=True)
                # dispatch send position: e*2*BCAP + (group within slice)*BCAP
                dbase_i = rp.tile([P, NT_SL, E], I32, name="dbi")
                nc.gpsimd.iota(dbase_i.rearrange("p (g j) e -> p g j e", j=4),
                               pattern=[[BCAP, 2], [0, 4], [2 * BCAP, E]],
                               base=0, channel_multiplier=0)
                posd = rp.tile([P, NT_SL, E], F32, name="posd")
                nc.vector.tensor_copy(out=posd[:], in_=dbase_i[:])
                nc.vector.tensor_tensor(
                    out=posd[:], in0=posd[:],
                    in1=ps_rank.rearrange("p (tt e) -> p tt e", e=E),
                    op=mybir.AluOpType.add)
                nc.vector.tensor_mul(posd[:], posd[:], m[:])
                p2d = rp.tile([P, NT_SL], F32, name="p2d")
                nc.vector.reduce_max(out=p2d[:], in_=posd[:], axis=mybir.AxisListType.X)
                p1d = rp.tile([P, NT_SL], F32, name="p1d")
                nc.vector.reduce_sum(out=p1d[:], in_=posd[:], axis=mybir.AxisListType.X)
                nc.vector.tensor_tensor(out=p1d[:], in0=p1d[:], in1=p2d[:],
                                        op=mybir.AluOpType.subtract)
                idx1 = rp.tile([P, NT_SL], I32, name="idx1")
                nc.vector.tensor_copy(out=idx1[:], in_=p1d[:])
                idx2 = rp.tile([P, NT_SL], I32, name="idx2")
                nc.vector.tensor_copy(out=idx2[:], in_=p2d[:])
                # per-token expert pair (e1 < e2) and their w values
                ei = rp.tile([P, NT_SL, E], I32, name="ei")
                nc.gpsimd.iota(ei[:], pattern=[[0, NT_SL], [1, E]], base=0,
                               channel_multiplier=0)
                eif = rp.tile([P, NT_SL, E], F32, name="eif")
                nc.vector.tensor_copy(out=eif[:], in_=ei[:])
                exm = rp.tile([P, NT_SL, E], F32, name="exm")
                nc.vector.tensor_mul(exm[:], eif[:], m[:])
                e2v = rp.tile([P, NT_SL], F32, name="e2v")
                nc.vector.reduce_max(out=e2v[:], in_=exm[:], axis=mybir.AxisListType.X)
                oh2 = rp.tile([P, NT_SL, E], F32, name="oh2")
                nc.vector.tensor_tensor(
                    out=oh2[:], in0=eif[:],
                    in1=e2v[:].unsqueeze(2).to_broadcast([P, NT_SL, E]),
                    op=mybir.AluOpType.is_equal)
                nc.vector.tensor_mul(oh2[:], oh2[:], wt_all[:])
                w2v = rp.tile([P, NT_SL], F32, name="w2v")
                nc.vector.reduce_sum(out=w2v[:], in_=oh2[:], axis=mybir.AxisListType.X)
                wsum = rp.tile([P, NT_SL], F32, name="wsum")
                nc.vector.reduce_sum(out=wsum[:], in_=wt_all[:], axis=mybir.AxisListType.X)
                w1v = rp.tile([P, NT_SL], F32, name="w1v")
                nc.vector.tensor_tensor(out=w1v[:], in0=wsum[:], in1=w2v[:],
                                        op=mybir.AluOpType.subtract)
                side1 = rp.tile([P, NT_SL, 2], F32, name="side1")
                nc.vector.tensor_copy(out=side1[:, :, 0], in_=w1v[:])
                nc.vector.tensor_copy(out=side1[:, :, 1], in_=gid_sb[:])
                side2 = rp.tile([P, NT_SL, 2], F32, name="side2")
                nc.vector.tensor_copy(out=side2[:, :, 0], in_=w2v[:])
                nc.vector.tensor_copy(out=side2[:, :, 1], in_=gid_sb[:])
                breg_d = nc.gpsimd.to_reg(2 * CAPF - 1)
                for tt in range(NT_SL):
                    for k, (ix, sd) in enumerate(((idx1, side1), (idx2, side2))):
                        st_ap = dspA_s[0:1, :]
                        st_ap = bass.AP(tensor=st_ap.tensor, offset=0, ap=st_ap.ap,
                                        dep_tracking_offset=(tt * 2 + k) * 2)
                        nc.gpsimd.indirect_dma_start(
                            out=st_ap,
                            out_offset=bass.IndirectOffsetOnAxis(ap=ix[:, tt:tt + 1], axis=0),
                            in_=sd[:, tt, :], in_offset=None,
                            bounds_check=breg_d, oob_is_err=False,
                        )
                nc.gpsimd.collective_compute(
                    "AllToAll", mybir.AluOpType.bypass, replica_groups=GROUPS,
                    ins=[dspA_s[:]], outs=[dspA_r[:]],
                )
                nc.gpsimd.collective_compute(
                    "AllGather", mybir.AluOpType.bypass, replica_groups=GROUPS,
                    ins=[w_sl[:]], outs=[w_all[:]],
                )

            # ---------------- phase F: expert FFN + A2A + combine ----------
            cpp = tc.tile_pool(name="cpers", bufs=1)
            cp = cpp.__enter__()
            i1 = [None, None]
            i2 = [None, None]
            fbig_cm = tc.tile_pool(name="fbig", bufs=2)
            fa_cm = tc.tile_pool(name="fa", bufs=1)
            fxg_cm = tc.tile_pool(name="fxg", bufs=12)
            fs_cm = tc.tile_pool(name="fsmall", bufs=2)
            fps_cm = tc.tile_pool(name="fpsum", bufs=1, space="PSUM")
            fbig = fbig_cm.__enter__()
            fa = fa_cm.__enter__()
            fxg = fxg_cm.__enter__()
            fs = fs_cm.__enter__()
            fps = fps_cm.__enter__()
            # issue both halves' side-table loads and x gathers up front so
            # half-1 prefetch overlaps half-0 compute
            wv = [None, None]
            xgs = [[], []]
            for h in range(2):
                sv = fs.tile([P, NST, 2], F32, name=f"sv{h}", tag="sv")
                nc.sync.dma_start(
                    out=sv[:],
                    in_=dspA_r[h * CAPF:(h + 1) * CAPF, :].rearrange("(st p) c -> p st c", p=P))
                wv[h] = fs.tile([P, NST], F32, name=f"wv{h}", tag="wv")
                nc.vector.tensor_copy(out=wv[h][:], in_=sv[:, :, 0])
                idg = fs.tile([P, NST], I32, name=f"idg{h}", tag="idg")
                nc.vector.tensor_copy(out=idg[:], in_=sv[:, :, 1])
                for st in range(NST):
                    xg = fxg.tile([P, C], BF16, name=f"xg_{h}_{st}", tag="xg")
                    nc.gpsimd.indirect_dma_start(
                        out=xg[:], out_offset=None,
                        in_=xbf_in[:],
                        in_offset=bass.IndirectOffsetOnAxis(ap=idg[:, st:st + 1], axis=0),
                    )
                    xgs[h].append(xg)

            # owner-side bucket positions (for combine; emitted mid-h0 so its
            # PE matmuls don't stall the in-order PE queue on the AllGather)
            def emit_owner_ranks(dpool, dps):
                oidx = dpool.tile([P, 2], I32)
                nc.sync.dma_start(out=oidx[:], in_=oidx_in[:])
                ebase_i = dpool.tile([P, E], I32, name="ebi")
                nc.gpsimd.iota(ebase_i[:], pattern=[[BCAP, E]], base=0,
                               channel_multiplier=0)
                ebase_f = dpool.tile([P, E], F32, name="ebf")
                nc.vector.tensor_copy(out=ebase_f[:], in_=ebase_i[:])
                w_all2 = w_all.rearrange("r (b f) -> (r b) f", b=2)
                for h in range(2):
                    w4 = dpool.tile([P, 32], F32, name=f"w4_{h}", tag="w4")
                    nc.gpsimd.indirect_dma_start(
                        out=w4[:], out_offset=None,
                        in_=w_all2,
                        in_offset=bass.IndirectOffsetOnAxis(ap=oidx[:, h:h + 1], axis=0),
                    )
                    m4 = dpool.tile([P, 32], F32, name=f"m4_{h}", tag="m4")
                    nc.vector.tensor_scalar(m4[:], w4[:], 0.0, scalar2=None,
                                            op0=mybir.AluOpType.is_gt)
                    ms4 = dpool.tile([P, 32], F32, name=f"ms4_{h}", tag="ms4")
                    msv = ms4.rearrange("p (j e) -> p j e", e=E)
                    m4j = m4.rearrange("p (j e) -> p j e", e=E)
                    nc.vector.memset(msv[:, 0], 0.0)
                    nc.vector.tensor_copy(out=msv[:, 1], in_=m4j[:, 0])
                    nc.vector.tensor_add(msv[:, 2], msv[:, 1], m4j[:, 1])
                    nc.vector.tensor_add(msv[:, 3], msv[:, 2], m4j[:, 2])
                    ps_r4 = dps.tile([P, 32], F32, name=f"psr4_{h}", tag="trA")
                    nc.tensor.matmul(out=ps_r4[:], lhsT=tri128[:], rhs=m4[:],
                                     start=True, stop=False)
                    nc.tensor.matmul(out=ps_r4[:], lhsT=ones128[:], rhs=ms4[:],
                                     start=False, stop=True)
                    pos = dpool.tile([P, 4, E], F32, name=f"pos_{h}", tag="pos")
                    nc.vector.tensor_copy(out=pos[:], in_=ps_r4.rearrange("p (j e) -> p j e", e=E))
                    nc.vector.tensor_tensor(
                        out=pos[:], in0=pos[:],
                        in1=ebase_f[:].unsqueeze(1).to_broadcast([P, 4, E]),
                        op=mybir.AluOpType.add)
                    nc.vector.tensor_mul(pos[:], pos[:], m4j[:])
                    p2 = dpool.tile([P, 4], F32, name=f"p2_{h}", tag="p2")
                    nc.vector.reduce_max(out=p2[:], in_=pos[:], axis=mybir.AxisListType.X)
                    p1 = dpool.tile([P, 4], F32, name=f"p1_{h}", tag="p1")
                    nc.vector.reduce_sum(out=p1[:], in_=pos[:], axis=mybir.AxisListType.X)
                    nc.vector.tensor_tensor(out=p1[:], in0=p1[:], in1=p2[:],
                                            op=mybir.AluOpType.subtract)
                    i1[h] = cp.tile([P, 4], I32, name=f"i1_{h}")
                    nc.vector.tensor_copy(out=i1[h][:], in_=p1[:])
                    i2[h] = cp.tile([P, 4], I32, name=f"i2_{h}")
                    nc.vector.tensor_copy(out=i2[h][:], in_=p2[:])

            if True:
                for h in range(2):
                    xgT = fbig.tile([P, CC, CAPF], BF16, name=f"xgT{h}", tag="big")
                    for st in range(NST):
                        xg = xgs[h][st]
                        for cc in range(CC):
                            ps_t = fps.tile([P, P], BF16, name=f"ft{h}_{st}_{cc}",
                                            tag="trA" if (st * CC + cc) % 2 == 0 else "trB")
                            nc.tensor.transpose(out=ps_t[:], in_=xg[:, cc * P:(cc + 1) * P],
                                                identity=identb[:])
                            nc.vector.tensor_copy(out=xgT[:, cc, st * P:(st + 1) * P],
                                                  in_=ps_t[:])
                    # L1
                    a_t = fa.tile([P, CC, CAPF], BF16, name=f"a{h}", tag="abuf")
                    for ic in range(CC):
                        ps_h = [fps.tile([P, 512], F32, name=f"psh{h}_{ic}_{b}",
                                         tag=f"mmA{b}") for b in range(len(BLOCKS))]
                        for cc in range(CC):
                            for b, bw in enumerate(BLOCKS):
                                nc.tensor.matmul(out=ps_h[b][:, :bw],
                                                 lhsT=w1sb[:, cc, ic * P:(ic + 1) * P],
                                                 rhs=xgT[:, cc, BOFF[b]:BOFF[b] + bw],
                                                 start=(cc == 0), stop=(cc == CC - 1))
                        sil = fs.tile([P, CAPF], BF16, name=f"sil{h}_{ic}", tag="sil")
                        for b, bw in enumerate(BLOCKS):
                            nc.scalar.activation(sil[:, BOFF[b]:BOFF[b] + bw], ps_h[b][:, :bw],
                                                 ACTF.Silu, bias=b1_sb[:, ic:ic + 1])
                        ps_g = [fps.tile([P, 512], F32, name=f"psg{h}_{ic}_{b}",
                                         tag=f"mmB{b}") for b in range(len(BLOCKS))]
                        for cc in range(CC):
                            for b, bw in enumerate(BLOCKS):
                                nc.tensor.matmul(out=ps_g[b][:, :bw],
                                                 lhsT=wgsb[:, cc, ic * P:(ic + 1) * P],
                                                 rhs=xgT[:, cc, BOFF[b]:BOFF[b] + bw],
                                                 start=(cc == 0), stop=(cc == CC - 1))
                        g_sb = fs.tile([P, CAPF], BF16, name=f"g{h}_{ic}", tag="gsb")
                        for b, bw in enumerate(BLOCKS):
                            nc.scalar.activation(g_sb[:, BOFF[b]:BOFF[b] + bw], ps_g[b][:, :bw],
                                                 ACTF.Identity, bias=bg_sb[:, ic:ic + 1])
                        nc.vector.tensor_mul(a_t[:, ic], sil[:], g_sb[:])

                    if h == 0:
                        with tc.tile_pool(name="dpool", bufs=1) as dpool:
                            emit_owner_ranks(dpool, fps)

                    # L2
                    y_tok = fbig.tile([P, NST, C], BF16, name=f"ytok{h}", tag="big")
                    for mc in range(CC):
                        ps_y = [fps.tile([P, 512], F32, name=f"psy{h}_{mc}_{b}",
                                         tag=f"mm{'A' if mc % 2 == 0 else 'B'}{b}")
                                for b in range(len(BLOCKS))]
                        for ic in range(CC):
                            for b, bw in enumerate(BLOCKS):
                                nc.tensor.matmul(out=ps_y[b][:, :bw],
                                                 lhsT=w2sb[:, ic, mc * P:(mc + 1) * P],
                                                 rhs=a_t[:, ic, BOFF[b]:BOFF[b] + bw],
                                                 start=(ic == 0), stop=(ic == CC - 1))
                        y_sb = fs.tile([P, CAPF], BF16, name=f"ysb{h}_{mc}", tag="ysb")
                        for b, bw in enumerate(BLOCKS):
                            nc.scalar.activation(y_sb[:, BOFF[b]:BOFF[b] + bw], ps_y[b][:, :bw],
                                                 ACTF.Identity, bias=b2_sb[:, mc:mc + 1])
                        for st in range(NST):
                            ps_t2 = fps.tile([P, P], BF16, name=f"bt{h}_{mc}_{st}",
                                             tag="trA" if (st + mc) % 2 == 0 else "trB")
                            nc.tensor.transpose(out=ps_t2[:],
                                                in_=y_sb[:, st * P:(st + 1) * P],
                                                identity=identb[:])
                            # fused w-scale on the PSUM eviction (partitions
                            # are tokens after the transpose)
                            nc.vector.tensor_scalar_mul(
                                y_tok[:, st, mc * P:(mc + 1) * P], ps_t2[:],
                                wv[h][:, st:st + 1])
                    for st in range(NST):
                        nc.sync.dma_start(out=a2a_send[h][st * P:(st + 1) * P, :],
                                          in_=y_tok[:, st])
                    nc.gpsimd.collective_compute(
                        "AllToAll", mybir.AluOpType.bypass, replica_groups=GROUPS,
                        ins=[a2a_send[h][:]], outs=[a2a_recv[h][:]],
                    )
                    # combine this half while the next half computes
                    for j in range(4):
                        g1 = fs.tile([P, C], BF16, name=f"cg1_{h}_{j}", tag="cg", bufs=4)
                        nc.gpsimd.indirect_dma_start(
                            out=g1[:], out_offset=None,
                            in_=a2a_recv[h][:],
                            in_offset=bass.IndirectOffsetOnAxis(ap=i1[h][:, j:j + 1], axis=0),
                        )
                        g2 = fs.tile([P, C], BF16, name=f"cg2_{h}_{j}", tag="cg", bufs=4)
                        nc.gpsimd.indirect_dma_start(
                            out=g2[:], out_offset=None,
                            in_=a2a_recv[h][:],
                            in_offset=bass.IndirectOffsetOnAxis(ap=i2[h][:, j:j + 1], axis=0),
                        )
                        ot = fs.tile([P, C], F32, name=f"ot_{h}_{j}", tag="ot")
                        nc.vector.tensor_tensor(out=ot[:], in0=g1[:], in1=g2[:],
                                                op=mybir.AluOpType.add)
                        nc.sync.dma_start(
                            out=y_out[h * 512 + j * P:h * 512 + (j + 1) * P, :], in_=ot[:])
            for cm in (fps_cm, fs_cm, fxg_cm, fa_cm, fbig_cm, cpp, wpool):
                cm.__exit__(None, None, None)

    split_multi_waits(nc)
    return nc


_NC_CACHE = None


def _get_nc():
    global _NC_CACHE
    if _NC_CACHE is None:
        _NC_CACHE = build_nc()
    return _NC_CACHE


def _in_maps(inputs):
    bf16 = ml_dtypes.bfloat16
    x = np.ascontiguousarray(np.asarray(inputs["x"], dtype=np.float32).reshape(N, C))
    xbf = np.ascontiguousarray(x.astype(bf16))
    Wr = np.ascontiguousarray(np.asarray(inputs["Wr"], dtype=np.float32))
    br = np.ascontiguousarray(np.asarray(inputs["br"], dtype=np.float32))
    W1 = np.asarray(inputs["W1"], dtype=np.float32)
    b1 = np.asarray(inputs["b1"], dtype=np.float32)
    Wg = np.asarray(inputs["Wg"], dtype=np.float32)
    bg = np.asarray(inputs["bg"], dtype=np.float32)
    W2 = np.asarray(inputs["W2"], dtype=np.float32)
    b2 = np.asarray(inputs["b2"], dtype=np.float32)
    maps = []
    for c in range(N_CORES):
        ownidx = np.zeros((P, 2), np.int32)
        for h in range(2):
            r = 4 * h + c // 2
            ownidx[:, h] = (r * P + np.arange(P)) * 2 + (c % 2)
        gidsl = (c * SLICE + np.arange(SLICE)).reshape(NT_SL, P).T.astype(np.float32)
        maps.append({
            "xbf": xbf,
            "xslT": np.ascontiguousarray(x[c * SLICE:(c + 1) * SLICE].T),
            "wr": Wr, "br": br, "gidsl": np.ascontiguousarray(gidsl),
            "ownidx": ownidx,
            "w1": np.ascontiguousarray(W1[c].astype(bf16)),
            "b1": np.ascontiguousarray(b1[c]),
            "wg": np.ascontiguousarray(Wg[c].astype(bf16)),
            "bg": np.ascontiguousarray(bg[c]),
            "w2": np.ascontiguousarray(W2[c].astype(bf16)),
            "b2": np.ascontiguousarray(b2[c]),
        })
    return maps


def _assemble(results):
    # core c's y_slice = [half0 rows c*512:(c+1)*512, half1 rows ...]
    out = np.empty((N, C), np.float32)
    HS = 512
    for c in range(N_CORES):
        ys = results[c]["y_slice"]
        out[c * HS:(c + 1) * HS] = ys[:HS]
        out[HALF + c * HS:HALF + (c + 1) * HS] = ys[HS:]
    return out


def _run(inputs, trace=False):
    from concourse.bass_utils import run_bass_kernel_spmd

    nc = _get_nc()
    res = run_bass_kernel_spmd(nc, _in_maps(inputs), list(range(N_CORES)), trace=trace)
    out = _assemble(res.results)
    return out.reshape(B, T, C), res


def kernel(**inputs) -> np.ndarray:
    out, _ = _run(inputs, trace=False)
    return out
